# revision 42
# baseline (speedup 1.0000x reference)
"""Trainium2 Bass kernel for DeepProteinClassifier.

Contract: kernel(**inputs) takes the FULL unsharded inputs and returns
the FULL [32, 10] float32 output.

Sharding: data-parallel over batch B=32 across 8 NeuronCores (4 samples
per core); all weights replicated.

Optimizations over the naive formulation:
- Mask compaction: the output only depends on positions with mask==1
  (masked keys get exp(-1e9)=0 weight; the mean-pool zeroes masked
  queries). Each sample's ~487..543 kept positions are compacted and
  zero-padded to SP=640 (5 tiles of 128), cutting all attention-side
  work ~1.6x with bit-identical math for kept positions.
- Fused QK: scores = Q.K^T/sqrt(D) = x M x^T + (x.Wk^T bq)_k + c_q + c
  with M = Wq^T Wk / sqrt(D). Per-query constants cancel in softmax;
  the per-key term is a host-computed bias folded into the exp bias.
  This deletes one full 960x960 projection and both Q/K bias drains.
- fp8 (e4m3) matmuls in DoubleRow mode (2 contraction chunks per
  instruction = 2x PE throughput), fp32 PSUM accumulation. M is scaled
  by 1024 and Wv by 16 on host to stay in e4m3 normal range; scales are
  undone in the ACT drains. Softmax/LN statistics stay fp32, context
  and residual bf16.
- Contraction dim padded 960->1024 so all chunks are full 128 rows
  (4 DoubleRow pairs).

Per-core pipeline per sample (PE work interleaved so drains hide):
  T1T[j,q] = M^T-chunks @ xT-chunks (fp8 DR) -> DVE drain to fp8
  V[k,d|1] = xT-chunks^T @ Wv-chunks (fp8 DR) -> ACT drain (1/16) fp8
  ST[k,q]  = xT^T-chunks @ T1T-chunks (fp8 DR); ET = exp(ST/1024 + bias)
  CTX[q,d|r] = ET-chunks^T @ V-chunks (fp8 DR + 1 plain chunk)
  H = CTX/r + (x+bv); LN-stats; masked pool as PE matvec with
  alpha = mask/summask * rsqrt(var+eps); then 4-layer MLP (bf16).
"""

import numpy as np
import ml_dtypes

B, S, D = 32, 1024, 960
NCORES = 8
BPC = B // NCORES   # 4 samples per core
SP = 640            # kept positions per sample, padded (5 tiles of 128)
NKT = 5             # number of 128-row kept-position tiles
DP = 1024           # padded contraction dim (8 chunks of 128, 4 DR pairs)
NDC = 8             # number of 128-row d chunks
PD = 120            # MLP-side partition size (960 = 8*120)
LN_EPS = 1e-5
SC_M = 1024.0       # host scale on M (undone in exp drain)
SC_V = 16.0         # host scale on Wv (undone in V drain)
BF16 = ml_dtypes.bfloat16
F8 = ml_dtypes.float8_e4m3

_CACHE = {}


def _build_nc():
    import concourse.tile as tile
    from concourse import bacc, mybir

    class _Bacc(bacc.Bacc):
        """Bacc with the ACT table chooser steered to the combined
        ln+exp function set, so the per-sample Ln/Exp LayerNorm pair and
        the ET exp share ONE table (no per-sample ACT_TABLE_LOAD thrash)."""

        def insert_act_table_loads(self):
            import bass_rust as _bass_rust
            from concourse.hw_specs import get_activation_tables

            has_activation = any(
                isinstance(i, mybir.InstActivation)
                for b in self.main_func.blocks
                for i in b.instructions
            )
            if not has_activation:
                return
            tables = list(get_activation_tables(self.m.arch).items())
            combo = next(
                (f for n, f in tables if n == "natural_log_exp_and_others"), None
            )
            if combo is not None:
                tables = [
                    (n, f if n == "natural_log_exp_and_others" else f - combo)
                    for n, f in tables
                ]
            _bass_rust.insert_act_table_loads(self, tables)

    f32 = mybir.dt.float32
    bf16 = mybir.dt.bfloat16
    f8 = mybir.dt.float8e4
    Alu = mybir.AluOpType
    Act = mybir.ActivationFunctionType
    DR = mybir.MatmulPerfMode.DoubleRow

    nc = _Bacc("TRN2", target_bir_lowering=False, debug=False)

    # ---- DRAM parameters (per-core shard) ----
    xt_h = nc.declare_dram_parameter("xt", [BPC, DP, SP], f8, isOutput=False)
    xn_h = nc.declare_dram_parameter("xn", [BPC, SP, D], bf16, isOutput=False)
    xs_h = nc.declare_dram_parameter("xs", [BPC, 128, NKT], f32, isOutput=False)
    mnp_h = nc.declare_dram_parameter("mnp", [BPC, 128, NKT], f32, isOutput=False)
    mfs_h = nc.declare_dram_parameter("mfs", [BPC, 128, NKT], f32, isOutput=False)
    m8_h = nc.declare_dram_parameter("m8", [DP, DP], f8, isOutput=False)
    # wv col 960 = rowsums of Wv^T: the V projection then also emits
    # vsum_k = sum_d V[k,d], so sum_d ctx falls out of the ctx matmul as an
    # extra PSUM column (no ACT accumulators needed for the LN mean)
    wv_h = nc.declare_dram_parameter("wv", [DP, D + 1], f8, isOutput=False)
    # w1 rows 0:960 = (W1*ln_g)^T, row 960 = -(W1*ln_g)@ones (the pooled-mu
    # rank-1 correction rides the contraction), rows 961:1024 = 0
    w1_h = nc.declare_dram_parameter("w1", [DP, 512], bf16, isOutput=False)
    w2_h = nc.declare_dram_parameter("w2", [512, 256], bf16, isOutput=False)
    w3_h = nc.declare_dram_parameter("w3", [256, 128], bf16, isOutput=False)
    w4_h = nc.declare_dram_parameter("w4", [128, 10], bf16, isOutput=False)
    b1_h = nc.declare_dram_parameter("b1", [128, 4], f32, isOutput=False)
    b2_h = nc.declare_dram_parameter("b2", [128, 2], f32, isOutput=False)
    b3_h = nc.declare_dram_parameter("b3", [128, 1], f32, isOutput=False)
    b4_h = nc.declare_dram_parameter("b4", [10, 1], f32, isOutput=False)
    out_h = nc.declare_dram_parameter("out", [10, BPC], f32, isOutput=True)

    with tile.TileContext(nc) as tc:
        with (
            tc.tile_pool(name="wpool", bufs=1) as wpool,
            tc.tile_pool(name="xpool", bufs=2) as xpool,
            tc.tile_pool(name="big", bufs=2) as big,
            tc.tile_pool(name="stats", bufs=2) as stats,
            tc.tile_pool(name="psum", bufs=7, space="PSUM") as psum,
        ):
            def load_sample(j, defer=False):
                xt_sb = xpool.tile([128, NDC, SP], f8, tag="xt", name=f"xt{j}")
                if defer:
                    # pair-granular so the first T1T matmuls start ~1.5us in
                    for p in range(4):
                        nc.sync.dma_start(
                            xt_sb[:, 2 * p : 2 * p + 2, :],
                            xt_h[j, 256 * p : 256 * (p + 1)].rearrange(
                                "(c p) s -> p c s", p=128
                            ),
                        )
                else:
                    nc.sync.dma_start(
                        xt_sb[:], xt_h[j].rearrange("(c p) s -> p c s", p=128)
                    )
                xn_sb = xpool.tile([128, NKT, D], bf16, tag="xn", name=f"xn{j}")
                xs_sb = stats.tile([128, NKT], f32, tag="xs", name=f"xs{j}")
                mnp_sb = stats.tile([128, NKT], f32, tag="mnp", name=f"mnp{j}")
                mfs_sb = stats.tile([128, NKT], f32, tag="mfs", name=f"mfs{j}")
                if not defer:
                    nc.sync.dma_start(
                        xn_sb[:], xn_h[j].rearrange("(t p) d -> p t d", p=128)
                    )
                    nc.sync.dma_start(xs_sb[:], xs_h[j])
                    nc.sync.dma_start(mnp_sb[:], mnp_h[j])
                    nc.sync.dma_start(mfs_sb[:], mfs_h[j])
                return xt_sb, xn_sb, xs_sb, mnp_sb, mfs_sb

            # xt0 + attention weights first (they gate the first matmuls);
            # weight DMAs split in halves matching first-consumer slices so
            # compute starts as soon as each half lands. xn0/stats0 after.
            sample0 = load_sample(0, defer=True)
            # weights fetched on the Activation HWDGE queue, in parallel with
            # the sample loads on the sync queue
            m8_sb = wpool.tile([128, NDC, DP], f8)
            nc.scalar.dma_start(
                m8_sb[:, :, 0:256],
                m8_h[:, 0:256].rearrange("(c p) n -> p c n", p=128),
            )
            nc.scalar.dma_start(
                m8_sb[:, :, 256:512],
                m8_h[:, 256:512].rearrange("(c p) n -> p c n", p=128),
            )
            nc.scalar.dma_start(
                m8_sb[:, :, 512:DP],
                m8_h[:, 512:DP].rearrange("(c p) n -> p c n", p=128),
            )
            wv_sb = wpool.tile([128, NDC, DP], f8)
            nc.scalar.dma_start(
                wv_sb[:, :, 0:512],
                wv_h[:, 0:512].rearrange("(c p) n -> p c n", p=128),
            )
            nc.scalar.dma_start(
                wv_sb[:, :, 512 : D + 1],
                wv_h[:, 512 : D + 1].rearrange("(c p) n -> p c n", p=128),
            )
            nc.sync.dma_start(
                sample0[1][:], xn_h[0].rearrange("(t p) d -> p t d", p=128)
            )
            nc.sync.dma_start(sample0[2][:], xs_h[0])
            nc.sync.dma_start(sample0[3][:], mnp_h[0])
            nc.sync.dma_start(sample0[4][:], mfs_h[0])
            pooledT = wpool.tile([128, NDC, BPC], bf16)
            nc.vector.memset(pooledT[:], 0.0)
            ones4 = wpool.tile([4, 1], bf16)
            nc.vector.memset(ones4[:], 1.0)
            mlp_w = {}

            def load_mlp_weights():
                w1_sb = wpool.tile([128, NDC, 512], bf16, name="w1_sb")
                nc.sync.dma_start(w1_sb[:], w1_h[:].rearrange("(c p) n -> p c n", p=128))
                w2_sb = wpool.tile([128, 4, 256], bf16, name="w2_sb")
                nc.sync.dma_start(w2_sb[:], w2_h[:].rearrange("(c p) n -> p c n", p=128))
                w3_sb = wpool.tile([128, 2, 128], bf16, name="w3_sb")
                nc.sync.dma_start(w3_sb[:], w3_h[:].rearrange("(c p) n -> p c n", p=128))
                w4_sb = wpool.tile([128, 10], bf16, name="w4_sb")
                nc.sync.dma_start(w4_sb[:], w4_h[:])
                b1_sb = wpool.tile([128, 4], f32, name="b1_sb")
                nc.sync.dma_start(b1_sb[:], b1_h[:])
                b2_sb = wpool.tile([128, 2], f32, name="b2_sb")
                nc.sync.dma_start(b2_sb[:], b2_h[:])
                b3_sb = wpool.tile([128, 1], f32, name="b3_sb")
                nc.sync.dma_start(b3_sb[:], b3_h[:])
                b4_sb = wpool.tile([10, 1], f32, name="b4_sb")
                nc.sync.dma_start(b4_sb[:], b4_h[:])
                mlp_w.update(w1=w1_sb, w2=w2_sb, w3=w3_sb, w4=w4_sb,
                             b1=b1_sb, b2=b2_sb, b3=b3_sb, b4=b4_sb)

            pending_pool = None

            for j in range(BPC):
                if j == 0:
                    xt_sb, xn_sb, xs_sb, mnp_sb, mfs_sb = sample0
                else:
                    xt_sb, xn_sb, xs_sb, mnp_sb, mfs_sb = load_sample(j)
                if j == 1:
                    load_mlp_weights()

                # ---- T1T = M^T-chunks @ xT-chunks: [do(1024), q(640)] fp8 ----
                T1T = big.tile([128, NDC, SP], f8, tag="T1T", name=f"T1T{j}")
                with nc.named_scope(f"s{j}_t1"):
                    for t in range(NDC):
                        psA = psum.tile([128, 512], f32, tag="mm", name="pt1a")
                        psB = psum.tile([128, 512], f32, tag="mm", name="pt1b")
                        for p in range(4):
                            lw = m8_sb[:, 2 * p : 2 * p + 2, t * 128 : (t + 1) * 128]
                            nc.tensor.matmul(
                                psA[:], lhsT=lw,
                                rhs=xt_sb[:, 2 * p : 2 * p + 2, 0:512],
                                start=(p == 0), stop=(p == 3), perf_mode=DR,
                            )
                            nc.tensor.matmul(
                                psB[:, 0:128], lhsT=lw,
                                rhs=xt_sb[:, 2 * p : 2 * p + 2, 512:SP],
                                start=(p == 0), stop=(p == 3), perf_mode=DR,
                            )
                        nc.vector.tensor_copy(T1T[:, t, 0:512], psA[:])
                        nc.vector.tensor_copy(T1T[:, t, 512:SP], psB[:, 0:128])

                # ---- V in natural layout [k(640) on partitions, d|1 free];
                #      free dim padded to 1024 for aligned DR streaming ----
                V = big.tile([128, NKT, 1024], f8, tag="V", name=f"V{j}")
                nc.vector.memset(V[:, :, D + 1 : D + 2], 1.0)
                with nc.named_scope(f"s{j}_v"):
                    for st in range(NKT):
                        psA = psum.tile([128, 512], f32, tag="mm", name="psva")
                        psB = psum.tile([128, 512], f32, tag="mm", name="psvb")
                        for p in range(4):
                            lx = xt_sb[:, 2 * p : 2 * p + 2, st * 128 : (st + 1) * 128]
                            nc.tensor.matmul(
                                psA[:], lhsT=lx,
                                rhs=wv_sb[:, 2 * p : 2 * p + 2, 0:512],
                                start=(p == 0), stop=(p == 3), perf_mode=DR,
                            )
                            nc.tensor.matmul(
                                psB[:, 0:449], lhsT=lx,
                                rhs=wv_sb[:, 2 * p : 2 * p + 2, 512 : D + 1],
                                start=(p == 0), stop=(p == 3), perf_mode=DR,
                            )
                        nc.vector.tensor_scalar_mul(
                            V[:, st, 0:512], psA[:], 1.0 / SC_V
                        )
                        nc.vector.tensor_scalar_mul(
                            V[:, st, 512 : D + 1], psB[:, 0:449], 1.0 / SC_V
                        )

                # ---- ST = xT^T @ T1T; ET = exp(ST/1024 + keybias) fp8 ----
                ET = big.tile([128, NKT, SP], f8, tag="ET", name=f"ET{j}")
                with nc.named_scope(f"s{j}_st"):
                    for kt in range(NKT):
                        psA = psum.tile([128, 512], f32, tag="mm", name="pssa")
                        psB = psum.tile([128, 512], f32, tag="mm", name="pssb")
                        for p in range(4):
                            lx = xt_sb[:, 2 * p : 2 * p + 2, kt * 128 : (kt + 1) * 128]
                            nc.tensor.matmul(
                                psA[:], lhsT=lx,
                                rhs=T1T[:, 2 * p : 2 * p + 2, 0:512],
                                start=(p == 0), stop=(p == 3), perf_mode=DR,
                            )
                            nc.tensor.matmul(
                                psB[:, 0:128], lhsT=lx,
                                rhs=T1T[:, 2 * p : 2 * p + 2, 512:SP],
                                start=(p == 0), stop=(p == 3), perf_mode=DR,
                            )
                        nc.scalar.activation(
                            ET[:, kt, 0:512], psA[:], Act.Exp,
                            bias=mnp_sb[:, kt : kt + 1], scale=1.0 / SC_M,
                        )
                        nc.scalar.activation(
                            ET[:, kt, 512:SP], psB[:, 0:128], Act.Exp,
                            bias=mnp_sb[:, kt : kt + 1], scale=1.0 / SC_M,
                        )

                # previous sample's pool matvec lands here: its AL/H are long
                # ready, and it fills the PE while the ET exp drains finish
                if pending_pool is not None:
                    pending_pool()
                    pending_pool = None

                # ---- context + residual + per-tile LN stats (LayerNorm is
                #      per-row, so tile qt's alpha is ready as soon as its
                #      context drains -- the pool matvec pipelines per-tile) --
                H = big.tile([128, NKT, 1024], bf16, tag="H", name=f"H{j}")
                SQ = stats.tile([128, NKT], f32, tag="SQ", name=f"SQ{j}")
                recips = stats.tile([128, NKT], f32, tag="recips", name=f"rc{j}")
                MU = stats.tile([128, NKT], f32, tag="MU", name=f"MU{j}")
                VAR = stats.tile([128, NKT], f32, tag="VAR", name=f"VAR{j}")
                RS = stats.tile([128, NKT], f32, tag="RS", name=f"RS{j}")
                # alpha split into 4 partition-quarter columns: the pool
                # matmul then runs at M=4 (full streaming rate, unlike M=1)
                AL4 = stats.tile([128, NKT, 4], bf16, tag="AL4", name=f"AL4{j}")
                nc.vector.memset(AL4[:], 0.0)
                with nc.named_scope(f"s{j}_ctx"):
                    for qt in range(NKT):
                        ps0 = psum.tile([128, 512], f32, tag="mm", name="psc0")
                        ps1 = psum.tile([128, 512], f32, tag="mm", name="psc1")
                        for p in range(2):
                            le = ET[:, 2 * p : 2 * p + 2, qt * 128 : (qt + 1) * 128]
                            nc.tensor.matmul(
                                ps0[:], lhsT=le,
                                rhs=V[:, 2 * p : 2 * p + 2, 0:512],
                                start=(p == 0), stop=False, perf_mode=DR,
                            )
                            nc.tensor.matmul(
                                ps1[:, 0:450], lhsT=le,
                                rhs=V[:, 2 * p : 2 * p + 2, 512 : D + 2],
                                start=(p == 0), stop=False, perf_mode=DR,
                            )
                        le = ET[:, 4, qt * 128 : (qt + 1) * 128]
                        nc.tensor.matmul(
                            ps0[:], lhsT=le, rhs=V[:, 4, 0:512],
                            start=False, stop=True,
                        )
                        nc.tensor.matmul(
                            ps1[:, 0:450], lhsT=le, rhs=V[:, 4, 512 : D + 2],
                            start=False, stop=True,
                        )
                        # col 449: r (softmax denom); col 448: sum_d ctx_d
                        q = slice(qt, qt + 1)
                        nc.vector.reciprocal(
                            recips[:, q], ps1[:, 449:450]
                        )
                        ctx0 = stats.tile([128, 512], bf16, tag="ctx0",
                                          name=f"c0_{j}_{qt}")
                        ctx1 = stats.tile([128, 448], bf16, tag="ctx1",
                                          name=f"c1_{j}_{qt}")
                        nc.scalar.activation(
                            ctx0[:], ps0[:], Act.Copy,
                            scale=recips[:, q],
                        )
                        nc.scalar.activation(
                            ctx1[:], ps1[:, 0:448], Act.Copy,
                            scale=recips[:, q],
                        )
                        # mu = (sum ctx + sum xn)/D, from the vsum PSUM column
                        nc.vector.tensor_tensor(MU[:, q], ps1[:, 448:449],
                                                recips[:, q], Alu.mult)
                        nc.vector.tensor_scalar(
                            MU[:, q], MU[:, q], xs_sb[:, q], 1.0 / D,
                            Alu.add, Alu.mult,
                        )
                        nc.vector.tensor_copy(H[:, qt, D : D + 1], MU[:, q])
                        nc.vector.tensor_add(
                            H[:, qt, 0:512], ctx0[:], xn_sb[:, qt, 0:512]
                        )
                        nc.vector.tensor_add(
                            H[:, qt, 512:D], ctx1[:], xn_sb[:, qt, 512:D]
                        )
                        scratch = stats.tile(
                            [128, D], bf16, tag="scr", name=f"scr{j}_{qt}", bufs=1
                        )
                        nc.scalar.activation(
                            scratch[:], H[:, qt, 0:D], Act.Square,
                            accum_out=SQ[:, qt : qt + 1],
                        )
                        # var = SQ/D + eps - mu^2 -> rs = exp(-0.5 ln var)
                        nc.vector.tensor_tensor(VAR[:, q], MU[:, q], MU[:, q],
                                                Alu.mult)
                        T2q = stats.tile([128, 1], f32, tag="T2",
                                         name=f"T2{j}_{qt}")
                        nc.vector.tensor_scalar(
                            T2q[:], SQ[:, q], 1.0 / D, LN_EPS,
                            Alu.mult, Alu.add,
                        )
                        nc.vector.tensor_sub(VAR[:, q], T2q[:], VAR[:, q])
                        nc.scalar.activation(VAR[:, q], VAR[:, q], Act.Ln)
                        nc.scalar.activation(RS[:, q], VAR[:, q], Act.Exp,
                                             scale=-0.5)
                        for m in range(4):
                            pr = slice(32 * m, 32 * (m + 1))
                            nc.vector.tensor_tensor(
                                AL4[pr, qt, m : m + 1], mfs_sb[pr, q],
                                RS[pr, q], Alu.mult,
                            )

                # ---- masked-mean pool: 4 partition-quarter partial pools
                #      [4, 961], then 8 tiny K=4 matvecs against ones to
                #      reduce AND transpose into pooledT[:, :, j] directly
                def emit_pool(j=j, AL4=AL4, H=H):
                    pp0 = psum.tile([128, 512], f32, tag="mm", name="pp0")
                    pp1 = psum.tile([128, 512], f32, tag="mm", name="pp1")
                    for c in range(NKT):
                        nc.tensor.matmul(
                            pp0[:4, :],
                            lhsT=AL4[:, c, :],
                            rhs=H[:, c, 0:512],
                            start=(c == 0), stop=(c == NKT - 1),
                        )
                        nc.tensor.matmul(
                            pp1[:4, 0:449],
                            lhsT=AL4[:, c, :],
                            rhs=H[:, c, 512 : D + 1],
                            start=(c == 0), stop=(c == NKT - 1),
                        )
                    pool4 = stats.tile([4, D + 1], bf16, tag="pool4",
                                       name=f"pool4_{j}", bufs=1)
                    nc.scalar.activation(pool4[:, 0:512], pp0[:4, :], Act.Copy)
                    nc.scalar.activation(
                        pool4[:, 512 : D + 1], pp1[:4, 0:449], Act.Copy
                    )
                    ptp = psum.tile([128, 512], f32, tag="mm", name=f"ptp{j}")
                    for w in range(NDC):
                        size = 128 if w < 7 else D + 1 - 7 * 128
                        nc.tensor.matmul(
                            ptp[:size, w : w + 1],
                            lhsT=pool4[:, w * 128 : w * 128 + size],
                            rhs=ones4[:],
                            start=True, stop=True,
                        )
                    # chunk 7 only has 65 valid rows (961 = 7*128 + 65); the
                    # rest of pooledT stays zero against w1's zero-padded rows
                    nc.scalar.activation(
                        pooledT[:, 0:7, j], ptp[:, 0:7], Act.Copy
                    )
                    nc.scalar.activation(
                        pooledT[0:65, 7, j : j + 1], ptp[0:65, 7:8], Act.Copy
                    )

                if j == BPC - 1:
                    # last sample: no next-sample matmuls to hide behind --
                    # emit inline so pool chunks interleave with the LN chain
                    emit_pool()
                else:
                    pending_pool = emit_pool

            # ---- MLP in transposed layout ----
            h1T = stats.tile([128, 4, BPC], bf16, tag="h1T")
            for m in range(4):
                ps = psum.tile([128, 512], f32, tag="mm", name=f"psm1{m}")
                for c in range(NDC):
                    nc.tensor.matmul(
                        ps[:, :BPC],
                        lhsT=mlp_w["w1"][:, c, m * 128 : (m + 1) * 128],
                        rhs=pooledT[:, c, :],
                        start=(c == 0), stop=(c == NDC - 1),
                    )
                nc.scalar.activation(
                    h1T[:, m, :], ps[:, :BPC], Act.Relu, bias=mlp_w["b1"][:, m : m + 1]
                )
            h2T = stats.tile([128, 2, BPC], bf16, tag="h2T")
            for m in range(2):
                ps = psum.tile([128, 512], f32, tag="mm", name=f"psm2{m}")
                for c in range(4):
                    nc.tensor.matmul(
                        ps[:, :BPC],
                        lhsT=mlp_w["w2"][:, c, m * 128 : (m + 1) * 128],
                        rhs=h1T[:, c, :],
                        start=(c == 0), stop=(c == 3),
                    )
                nc.scalar.activation(
                    h2T[:, m, :], ps[:, :BPC], Act.Relu, bias=mlp_w["b2"][:, m : m + 1]
                )
            h3T = stats.tile([128, 1, BPC], bf16, tag="h3T")
            ps = psum.tile([128, 512], f32, tag="mm", name="psm3")
            for c in range(2):
                nc.tensor.matmul(
                    ps[:, :BPC],
                    lhsT=mlp_w["w3"][:, c, :],
                    rhs=h2T[:, c, :],
                    start=(c == 0), stop=(c == 1),
                )
            nc.scalar.activation(
                h3T[:, 0, :], ps[:, :BPC], Act.Relu, bias=mlp_w["b3"][:, 0:1]
            )
            ps4 = psum.tile([128, 512], f32, tag="mm", name="psm4")
            nc.tensor.matmul(
                ps4[:10, :BPC], lhsT=mlp_w["w4"][:, :], rhs=h3T[:, 0, :],
                start=True, stop=True,
            )
            osb = stats.tile([10, BPC], f32, tag="osb")
            nc.scalar.activation(osb[:], ps4[:10, :BPC], Act.Identity, bias=mlp_w["b4"][:])
            nc.sync.dma_start(out_h[:], osb[:])

    nc.compile()
    return nc


def _get_nc():
    if "nc" not in _CACHE:
        _CACHE["nc"] = _build_nc()
    return _CACHE["nc"]


def host_prep(inputs):
    """Build the 8 per-core in_maps from the full inputs."""
    x = np.asarray(inputs["x"], np.float32)
    mask = np.asarray(inputs["mask"])
    Wq, bq = np.asarray(inputs["Wq"], np.float32), np.asarray(inputs["bq"], np.float32)
    Wk, bk = np.asarray(inputs["Wk"], np.float32), np.asarray(inputs["bk"], np.float32)
    Wv, bv = np.asarray(inputs["Wv"], np.float32), np.asarray(inputs["bv"], np.float32)
    ln_g, ln_b = np.asarray(inputs["ln_g"], np.float32), np.asarray(inputs["ln_b"], np.float32)
    W1, b1 = np.asarray(inputs["W1"], np.float32), np.asarray(inputs["b1"], np.float32)
    W2, b2 = np.asarray(inputs["W2"], np.float32), np.asarray(inputs["b2"], np.float32)
    W3, b3 = np.asarray(inputs["W3"], np.float32), np.asarray(inputs["b3"], np.float32)
    W4, b4 = np.asarray(inputs["W4"], np.float32), np.asarray(inputs["b4"], np.float32)

    isq = 1.0 / np.sqrt(np.float32(D))
    # fused QK matrix, scaled into e4m3 range
    M = (Wq.T @ Wk) * isq
    Mpad = np.zeros((DP, DP), np.float32)
    Mpad[:D, :D] = M * SC_M
    m8 = Mpad.astype(F8)
    # key-side score bias direction (bq . K_k term)
    u_k = (Wk.T @ bq) * isq
    wvp = np.zeros((DP, D + 1), np.float32)
    wvp[:D, :D] = Wv.T * SC_V
    wvp[:D, D] = (Wv.T * SC_V).sum(axis=1)   # vsum col -> sum_d V[k,d]
    wv8 = wvp.astype(F8)

    W1e = W1 * ln_g[None, :]
    b1e = b1 + W1 @ ln_b
    w1p = np.zeros((DP, 512), np.float32)
    w1p[:D] = W1e.T
    w1p[D] = -W1e.sum(axis=1)     # pooled-mu rank-1 correction row
    w1 = w1p.astype(BF16)
    b1p = np.ascontiguousarray(b1e.reshape(4, 128).T).astype(np.float32)
    w2 = np.ascontiguousarray(W2.T).astype(BF16)
    b2p = np.ascontiguousarray(b2.reshape(2, 128).T).astype(np.float32)
    w3 = np.ascontiguousarray(W3.T).astype(BF16)
    b3p = np.ascontiguousarray(b3.reshape(1, 128).T).astype(np.float32)
    w4 = np.ascontiguousarray(W4.T).astype(BF16)
    b4p = np.ascontiguousarray(b4.reshape(10, 1)).astype(np.float32)

    shared = dict(
        m8=m8, wv=wv8,
        w1=w1, w2=w2, w3=w3, w4=w4,
        b1=b1p, b2=b2p, b3=b3p, b4=b4p,
    )
    in_maps = []
    for core in range(NCORES):
        xt = np.zeros((BPC, DP, SP), F8)
        xn = np.zeros((BPC, SP, D), BF16)
        xs = np.zeros((BPC, 128, NKT), np.float32)
        mnp = np.full((BPC, 128, NKT), -1e9, np.float32)
        mfs = np.zeros((BPC, 128, NKT), np.float32)
        for jj in range(BPC):
            b = core * BPC + jj
            idx = np.nonzero(mask[b])[0]
            n = len(idx)
            assert n <= SP, f"sample {b}: {n} kept positions > SP={SP}"
            xk = x[b, idx]                                # [n, D]
            xt[jj, :D, :n] = xk.T.astype(F8)
            xnj = (xk + bv[None, :]).astype(BF16)
            xn[jj, :n] = xnj
            xsj = np.zeros(SP, np.float32)
            xsj[:n] = xnj.astype(np.float32).sum(axis=1)
            xs[jj] = xsj.reshape(NKT, 128).T
            mnpj = np.full(SP, -1e9, np.float32)
            mnpj[:n] = xk @ u_k
            mnp[jj] = mnpj.reshape(NKT, 128).T
            mfsj = np.zeros(SP, np.float32)
            mfsj[:n] = 1.0 / n
            mfs[jj] = mfsj.reshape(NKT, 128).T
        m = dict(shared)
        m.update(xt=xt, xn=xn, xs=xs, mnp=mnp, mfs=mfs)
        in_maps.append(m)
    return in_maps


def assemble(results):
    """results: list of 8 dicts with 'out' [10, BPC] -> [32, 10] f32."""
    return np.concatenate(
        [np.asarray(r["out"], np.float32).T for r in results], axis=0
    )


def kernel(**inputs):
    from concourse.bass_utils import run_bass_kernel_spmd

    nc = _get_nc()
    in_maps = host_prep(inputs)
    res = run_bass_kernel_spmd(nc, in_maps, core_ids=list(range(NCORES)))
    return assemble(res.results)


# revision 43
# speedup vs baseline: 1.0543x; 1.0543x over previous
"""Trainium2 Bass kernel for DeepProteinClassifier.

Contract: kernel(**inputs) takes the FULL unsharded inputs and returns
the FULL [32, 10] float32 output.

Sharding: data-parallel over batch B=32 across 8 NeuronCores (4 samples
per core); all weights replicated.

Optimizations over the naive formulation:
- Mask compaction: the output only depends on positions with mask==1
  (masked keys get exp(-1e9)=0 weight; the mean-pool zeroes masked
  queries). Each sample's ~487..543 kept positions are compacted and
  zero-padded to SP=640 (5 tiles of 128), cutting all attention-side
  work ~1.6x with bit-identical math for kept positions.
- Fused QK: scores = Q.K^T/sqrt(D) = x M x^T + (x.Wk^T bq)_k + c_q + c
  with M = Wq^T Wk / sqrt(D). Per-query constants cancel in softmax;
  the per-key term is a host-computed bias folded into the exp bias.
  This deletes one full 960x960 projection and both Q/K bias drains.
- fp8 (e4m3) matmuls in DoubleRow mode (2 contraction chunks per
  instruction = 2x PE throughput), fp32 PSUM accumulation. M is scaled
  by 1024 and Wv by 16 on host to stay in e4m3 normal range; scales are
  undone in the ACT drains. Softmax/LN statistics stay fp32, context
  and residual bf16.
- Contraction dim padded 960->1024 so all chunks are full 128 rows
  (4 DoubleRow pairs).

Per-core pipeline per sample (PE work interleaved so drains hide):
  T1T[j,q] = M^T-chunks @ xT-chunks (fp8 DR) -> DVE drain to fp8
  V[k,d|1] = xT-chunks^T @ Wv-chunks (fp8 DR) -> ACT drain (1/16) fp8
  ST[k,q]  = xT^T-chunks @ T1T-chunks (fp8 DR); ET = exp(ST/1024 + bias)
  CTX[q,d|r] = ET-chunks^T @ V-chunks (fp8 DR + 1 plain chunk)
  H = CTX/r + (x+bv); LN-stats; masked pool as PE matvec with
  alpha = mask/summask * rsqrt(var+eps); then 4-layer MLP (bf16).
"""

import numpy as np
import ml_dtypes

B, S, D = 32, 1024, 960
NCORES = 8
BPC = B // NCORES   # 4 samples per core
SP = 640            # kept positions per sample, padded (5 tiles of 128)
NKT = 5             # number of 128-row kept-position tiles
DP = 1024           # padded contraction dim (8 chunks of 128, 4 DR pairs)
NDC = 8             # number of 128-row d chunks
PD = 120            # MLP-side partition size (960 = 8*120)
LN_EPS = 1e-5
SC_M = 1024.0       # host scale on M (undone in exp drain)
SC_V = 16.0         # host scale on Wv (undone in V drain)
BF16 = ml_dtypes.bfloat16
F8 = ml_dtypes.float8_e4m3

_CACHE = {}


def _build_nc():
    import concourse.tile as tile
    from concourse import bacc, mybir

    class _Bacc(bacc.Bacc):
        """Bacc with the ACT table chooser steered to the combined
        ln+exp function set, so the per-sample Ln/Exp LayerNorm pair and
        the ET exp share ONE table (no per-sample ACT_TABLE_LOAD thrash)."""

        def insert_act_table_loads(self):
            import bass_rust as _bass_rust
            from concourse.hw_specs import get_activation_tables

            has_activation = any(
                isinstance(i, mybir.InstActivation)
                for b in self.main_func.blocks
                for i in b.instructions
            )
            if not has_activation:
                return
            tables = list(get_activation_tables(self.m.arch).items())
            combo = next(
                (f for n, f in tables if n == "natural_log_exp_and_others"), None
            )
            if combo is not None:
                tables = [
                    (n, f if n == "natural_log_exp_and_others" else f - combo)
                    for n, f in tables
                ]
            _bass_rust.insert_act_table_loads(self, tables)

    f32 = mybir.dt.float32
    bf16 = mybir.dt.bfloat16
    f8 = mybir.dt.float8e4
    Alu = mybir.AluOpType
    Act = mybir.ActivationFunctionType
    DR = mybir.MatmulPerfMode.DoubleRow

    nc = _Bacc("TRN2", target_bir_lowering=False, debug=False)

    # ---- DRAM parameters (per-core shard) ----
    xt_h = nc.declare_dram_parameter("xt", [BPC, DP, SP], f8, isOutput=False)
    xn_h = nc.declare_dram_parameter("xn", [BPC, SP, D], bf16, isOutput=False)
    xs_h = nc.declare_dram_parameter("xs", [BPC, 128, NKT], f32, isOutput=False)
    mnp_h = nc.declare_dram_parameter("mnp", [BPC, 128, NKT], f32, isOutput=False)
    mfs_h = nc.declare_dram_parameter("mfs", [BPC, 128, NKT], f32, isOutput=False)
    m8_h = nc.declare_dram_parameter("m8", [DP, DP], f8, isOutput=False)
    # wv col 960 = rowsums of Wv^T: the V projection then also emits
    # vsum_k = sum_d V[k,d], so sum_d ctx falls out of the ctx matmul as an
    # extra PSUM column (no ACT accumulators needed for the LN mean)
    wv_h = nc.declare_dram_parameter("wv", [DP, D + 1], f8, isOutput=False)
    w1s_h = nc.declare_dram_parameter("w1s", [1, 512], bf16, isOutput=False)
    w1_h = nc.declare_dram_parameter("w1", [D, 512], bf16, isOutput=False)
    w2_h = nc.declare_dram_parameter("w2", [512, 256], bf16, isOutput=False)
    w3_h = nc.declare_dram_parameter("w3", [256, 128], bf16, isOutput=False)
    w4_h = nc.declare_dram_parameter("w4", [128, 10], bf16, isOutput=False)
    b1_h = nc.declare_dram_parameter("b1", [128, 4], f32, isOutput=False)
    b2_h = nc.declare_dram_parameter("b2", [128, 2], f32, isOutput=False)
    b3_h = nc.declare_dram_parameter("b3", [128, 1], f32, isOutput=False)
    b4_h = nc.declare_dram_parameter("b4", [10, 1], f32, isOutput=False)
    id4_h = nc.declare_dram_parameter("id4", [4, 4], f32, isOutput=False)
    out_h = nc.declare_dram_parameter("out", [10, BPC], f32, isOutput=True)

    with tile.TileContext(nc) as tc:
        with (
            tc.tile_pool(name="wpool", bufs=1) as wpool,
            tc.tile_pool(name="xpool", bufs=2) as xpool,
            tc.tile_pool(name="big", bufs=2) as big,
            tc.tile_pool(name="stats", bufs=2) as stats,
            tc.tile_pool(name="psum", bufs=8, space="PSUM") as psum,
        ):
            def load_sample(j, defer=False):
                xt_sb = xpool.tile([128, NDC, SP], f8, tag="xt", name=f"xt{j}")
                if defer:
                    # pair-granular so the first T1T matmuls start ~1.5us in
                    for p in range(4):
                        nc.sync.dma_start(
                            xt_sb[:, 2 * p : 2 * p + 2, :],
                            xt_h[j, 256 * p : 256 * (p + 1)].rearrange(
                                "(c p) s -> p c s", p=128
                            ),
                        )
                else:
                    nc.sync.dma_start(
                        xt_sb[:], xt_h[j].rearrange("(c p) s -> p c s", p=128)
                    )
                xn_sb = xpool.tile([128, NKT, D], bf16, tag="xn", name=f"xn{j}")
                xs_sb = stats.tile([128, NKT], f32, tag="xs", name=f"xs{j}")
                mnp_sb = stats.tile([128, NKT], f32, tag="mnp", name=f"mnp{j}")
                mfs_sb = stats.tile([128, NKT], f32, tag="mfs", name=f"mfs{j}")
                if not defer:
                    nc.sync.dma_start(
                        xn_sb[:], xn_h[j].rearrange("(t p) d -> p t d", p=128)
                    )
                    nc.sync.dma_start(xs_sb[:], xs_h[j])
                    nc.sync.dma_start(mnp_sb[:], mnp_h[j])
                    nc.sync.dma_start(mfs_sb[:], mfs_h[j])
                return xt_sb, xn_sb, xs_sb, mnp_sb, mfs_sb

            # xt0 + attention weights first (they gate the first matmuls);
            # weight DMAs split in halves matching first-consumer slices so
            # compute starts as soon as each half lands. xn0/stats0 after.
            sample0 = load_sample(0, defer=True)
            # weights fetched on the Activation HWDGE queue, in parallel with
            # the sample loads on the sync queue
            m8_sb = wpool.tile([128, NDC, DP], f8)
            nc.scalar.dma_start(
                m8_sb[:, :, 0:256],
                m8_h[:, 0:256].rearrange("(c p) n -> p c n", p=128),
            )
            nc.scalar.dma_start(
                m8_sb[:, :, 256:512],
                m8_h[:, 256:512].rearrange("(c p) n -> p c n", p=128),
            )
            nc.scalar.dma_start(
                m8_sb[:, :, 512:DP],
                m8_h[:, 512:DP].rearrange("(c p) n -> p c n", p=128),
            )
            wv_sb = wpool.tile([128, NDC, DP], f8)
            nc.scalar.dma_start(
                wv_sb[:, :, 0:512],
                wv_h[:, 0:512].rearrange("(c p) n -> p c n", p=128),
            )
            nc.scalar.dma_start(
                wv_sb[:, :, 512 : D + 1],
                wv_h[:, 512 : D + 1].rearrange("(c p) n -> p c n", p=128),
            )
            nc.sync.dma_start(
                sample0[1][:], xn_h[0].rearrange("(t p) d -> p t d", p=128)
            )
            nc.sync.dma_start(sample0[2][:], xs_h[0])
            nc.sync.dma_start(sample0[3][:], mnp_h[0])
            nc.sync.dma_start(sample0[4][:], mfs_h[0])
            pooled_all = wpool.tile([BPC, D + 1], f32)
            murow = wpool.tile([1, BPC], bf16)
            mlp_w = {}

            def load_mlp_weights():
                w1s_sb = wpool.tile([1, 512], bf16, name="w1s_sb")
                nc.sync.dma_start(w1s_sb[:], w1s_h[:])
                mlp_w["w1s"] = w1s_sb
                w1_sb = wpool.tile([PD, NDC, 512], bf16, name="w1_sb")
                nc.sync.dma_start(w1_sb[:], w1_h[:].rearrange("(c p) n -> p c n", p=PD))
                w2_sb = wpool.tile([128, 4, 256], bf16, name="w2_sb")
                nc.sync.dma_start(w2_sb[:], w2_h[:].rearrange("(c p) n -> p c n", p=128))
                w3_sb = wpool.tile([128, 2, 128], bf16, name="w3_sb")
                nc.sync.dma_start(w3_sb[:], w3_h[:].rearrange("(c p) n -> p c n", p=128))
                w4_sb = wpool.tile([128, 10], bf16, name="w4_sb")
                nc.sync.dma_start(w4_sb[:], w4_h[:])
                b1_sb = wpool.tile([128, 4], f32, name="b1_sb")
                nc.sync.dma_start(b1_sb[:], b1_h[:])
                b2_sb = wpool.tile([128, 2], f32, name="b2_sb")
                nc.sync.dma_start(b2_sb[:], b2_h[:])
                b3_sb = wpool.tile([128, 1], f32, name="b3_sb")
                nc.sync.dma_start(b3_sb[:], b3_h[:])
                b4_sb = wpool.tile([10, 1], f32, name="b4_sb")
                nc.sync.dma_start(b4_sb[:], b4_h[:])
                id4_sb = wpool.tile([4, 4], f32, name="id4_sb")
                nc.sync.dma_start(id4_sb[:], id4_h[:])
                mlp_w.update(w1=w1_sb, w2=w2_sb, w3=w3_sb, w4=w4_sb,
                             b1=b1_sb, b2=b2_sb, b3=b3_sb, b4=b4_sb, id4=id4_sb)

            pending_pool = None

            for j in range(BPC):
                if j == 0:
                    xt_sb, xn_sb, xs_sb, mnp_sb, mfs_sb = sample0
                else:
                    xt_sb, xn_sb, xs_sb, mnp_sb, mfs_sb = load_sample(j)
                if j == 1:
                    load_mlp_weights()

                # ---- T1T = M^T-chunks @ xT-chunks: [do(1024), q(640)] fp8 ----
                T1T = big.tile([128, NDC, SP], f8, tag="T1T", name=f"T1T{j}")
                with nc.named_scope(f"s{j}_t1"):
                    for t in range(NDC):
                        psA = psum.tile([128, 512], f32, tag="mm", name="pt1a")
                        psB = psum.tile([128, 512], f32, tag="mm", name="pt1b")
                        for p in range(4):
                            lw = m8_sb[:, 2 * p : 2 * p + 2, t * 128 : (t + 1) * 128]
                            nc.tensor.matmul(
                                psA[:], lhsT=lw,
                                rhs=xt_sb[:, 2 * p : 2 * p + 2, 0:512],
                                start=(p == 0), stop=(p == 3), perf_mode=DR,
                            )
                            nc.tensor.matmul(
                                psB[:, 0:128], lhsT=lw,
                                rhs=xt_sb[:, 2 * p : 2 * p + 2, 512:SP],
                                start=(p == 0), stop=(p == 3), perf_mode=DR,
                            )
                        nc.vector.tensor_copy(T1T[:, t, 0:512], psA[:])
                        nc.vector.tensor_copy(T1T[:, t, 512:SP], psB[:, 0:128])

                # ---- V in natural layout [k(640) on partitions, d|1 free];
                #      free dim padded to 1024 for aligned DR streaming ----
                V = big.tile([128, NKT, 1024], f8, tag="V", name=f"V{j}")
                nc.vector.memset(V[:, :, D + 1 : D + 2], 1.0)
                with nc.named_scope(f"s{j}_v"):
                    for st in range(NKT):
                        psA = psum.tile([128, 512], f32, tag="mm", name="psva")
                        psB = psum.tile([128, 512], f32, tag="mm", name="psvb")
                        for p in range(4):
                            lx = xt_sb[:, 2 * p : 2 * p + 2, st * 128 : (st + 1) * 128]
                            nc.tensor.matmul(
                                psA[:], lhsT=lx,
                                rhs=wv_sb[:, 2 * p : 2 * p + 2, 0:512],
                                start=(p == 0), stop=(p == 3), perf_mode=DR,
                            )
                            nc.tensor.matmul(
                                psB[:, 0:449], lhsT=lx,
                                rhs=wv_sb[:, 2 * p : 2 * p + 2, 512 : D + 1],
                                start=(p == 0), stop=(p == 3), perf_mode=DR,
                            )
                        nc.vector.tensor_scalar_mul(
                            V[:, st, 0:512], psA[:], 1.0 / SC_V
                        )
                        nc.vector.tensor_scalar_mul(
                            V[:, st, 512 : D + 1], psB[:, 0:449], 1.0 / SC_V
                        )

                # ---- ST = xT^T @ T1T; ET = exp(ST/1024 + keybias) fp8 ----
                ET = big.tile([128, NKT, SP], f8, tag="ET", name=f"ET{j}")
                with nc.named_scope(f"s{j}_st"):
                    for kt in range(NKT):
                        psA = psum.tile([128, 512], f32, tag="mm", name="pssa")
                        psB = psum.tile([128, 512], f32, tag="mm", name="pssb")
                        for p in range(4):
                            lx = xt_sb[:, 2 * p : 2 * p + 2, kt * 128 : (kt + 1) * 128]
                            nc.tensor.matmul(
                                psA[:], lhsT=lx,
                                rhs=T1T[:, 2 * p : 2 * p + 2, 0:512],
                                start=(p == 0), stop=(p == 3), perf_mode=DR,
                            )
                            nc.tensor.matmul(
                                psB[:, 0:128], lhsT=lx,
                                rhs=T1T[:, 2 * p : 2 * p + 2, 512:SP],
                                start=(p == 0), stop=(p == 3), perf_mode=DR,
                            )
                        nc.scalar.activation(
                            ET[:, kt, 0:512], psA[:], Act.Exp,
                            bias=mnp_sb[:, kt : kt + 1], scale=1.0 / SC_M,
                        )
                        nc.scalar.activation(
                            ET[:, kt, 512:SP], psB[:, 0:128], Act.Exp,
                            bias=mnp_sb[:, kt : kt + 1], scale=1.0 / SC_M,
                        )

                # previous sample's pool matvec lands here: its AL/H are long
                # ready, and it fills the PE while the ET exp drains finish
                if pending_pool is not None:
                    pending_pool()
                    pending_pool = None

                # ---- context + residual + per-tile LN stats (LayerNorm is
                #      per-row, so tile qt's alpha is ready as soon as its
                #      context drains -- the pool matvec pipelines per-tile) --
                H = big.tile([128, NKT, 1024], bf16, tag="H", name=f"H{j}")
                SQ = stats.tile([128, NKT], f32, tag="SQ", name=f"SQ{j}")
                recips = stats.tile([128, NKT], f32, tag="recips", name=f"rc{j}")
                MU = stats.tile([128, NKT], f32, tag="MU", name=f"MU{j}")
                VAR = stats.tile([128, NKT], f32, tag="VAR", name=f"VAR{j}")
                RS = stats.tile([128, NKT], f32, tag="RS", name=f"RS{j}")
                AL = stats.tile([128, NKT], bf16, tag="AL", name=f"AL{j}")
                with nc.named_scope(f"s{j}_ctx"):
                    for qt in range(NKT):
                        ps0 = psum.tile([128, 512], f32, tag="mm", name="psc0")
                        ps1 = psum.tile([128, 512], f32, tag="mm", name="psc1")
                        for p in range(2):
                            le = ET[:, 2 * p : 2 * p + 2, qt * 128 : (qt + 1) * 128]
                            nc.tensor.matmul(
                                ps0[:], lhsT=le,
                                rhs=V[:, 2 * p : 2 * p + 2, 0:512],
                                start=(p == 0), stop=False, perf_mode=DR,
                            )
                            nc.tensor.matmul(
                                ps1[:, 0:450], lhsT=le,
                                rhs=V[:, 2 * p : 2 * p + 2, 512 : D + 2],
                                start=(p == 0), stop=False, perf_mode=DR,
                            )
                        le = ET[:, 4, qt * 128 : (qt + 1) * 128]
                        nc.tensor.matmul(
                            ps0[:], lhsT=le, rhs=V[:, 4, 0:512],
                            start=False, stop=True,
                        )
                        nc.tensor.matmul(
                            ps1[:, 0:450], lhsT=le, rhs=V[:, 4, 512 : D + 2],
                            start=False, stop=True,
                        )
                        # col 449: r (softmax denom); col 448: sum_d ctx_d
                        q = slice(qt, qt + 1)
                        nc.vector.reciprocal(
                            recips[:, q], ps1[:, 449:450]
                        )
                        ctx0 = stats.tile([128, 512], bf16, tag="ctx0",
                                          name=f"c0_{j}_{qt}")
                        ctx1 = stats.tile([128, 448], bf16, tag="ctx1",
                                          name=f"c1_{j}_{qt}")
                        nc.scalar.activation(
                            ctx0[:], ps0[:], Act.Copy,
                            scale=recips[:, q],
                        )
                        nc.scalar.activation(
                            ctx1[:], ps1[:, 0:448], Act.Copy,
                            scale=recips[:, q],
                        )
                        # mu = (sum ctx + sum xn)/D, from the vsum PSUM column
                        nc.vector.tensor_tensor(MU[:, q], ps1[:, 448:449],
                                                recips[:, q], Alu.mult)
                        nc.vector.tensor_scalar(
                            MU[:, q], MU[:, q], xs_sb[:, q], 1.0 / D,
                            Alu.add, Alu.mult,
                        )
                        nc.vector.tensor_copy(H[:, qt, D : D + 1], MU[:, q])
                        nc.vector.tensor_add(
                            H[:, qt, 0:512], ctx0[:], xn_sb[:, qt, 0:512]
                        )
                        nc.vector.tensor_add(
                            H[:, qt, 512:D], ctx1[:], xn_sb[:, qt, 512:D]
                        )
                        scratch = stats.tile(
                            [128, D], bf16, tag="scr", name=f"scr{j}_{qt}", bufs=1
                        )
                        nc.scalar.activation(
                            scratch[:], H[:, qt, 0:D], Act.Square,
                            accum_out=SQ[:, qt : qt + 1],
                        )
                        # var = SQ/D + eps - mu^2 -> rs = exp(-0.5 ln var)
                        nc.vector.tensor_tensor(VAR[:, q], MU[:, q], MU[:, q],
                                                Alu.mult)
                        T2q = stats.tile([128, 1], f32, tag="T2",
                                         name=f"T2{j}_{qt}")
                        nc.vector.tensor_scalar(
                            T2q[:], SQ[:, q], 1.0 / D, LN_EPS,
                            Alu.mult, Alu.add,
                        )
                        nc.vector.tensor_sub(VAR[:, q], T2q[:], VAR[:, q])
                        nc.scalar.activation(VAR[:, q], VAR[:, q], Act.Ln)
                        nc.scalar.activation(RS[:, q], VAR[:, q], Act.Exp,
                                             scale=-0.5)
                        nc.vector.tensor_tensor(AL[:, q], mfs_sb[:, q],
                                                RS[:, q], Alu.mult)

                # ---- masked-mean pool as PE matvec; each chunk c waits only
                #      on its own AL column so it pipelines with the LN chain
                def emit_pool(j=j, AL=AL, H=H):
                    pp0 = psum.tile([128, 512], f32, tag="mm", name="pp0")
                    pp1 = psum.tile([128, 512], f32, tag="mm", name="pp1")
                    for c in range(NKT):
                        nc.tensor.matmul(
                            pp0[:1, :],
                            lhsT=AL[:, c : c + 1],
                            rhs=H[:, c, 0:512],
                            start=(c == 0), stop=(c == NKT - 1),
                        )
                        nc.tensor.matmul(
                            pp1[:1, 0:449],
                            lhsT=AL[:, c : c + 1],
                            rhs=H[:, c, 512 : D + 1],
                            start=(c == 0), stop=(c == NKT - 1),
                        )
                    prow = stats.tile([1, D + 1], f32, tag="prow",
                                      name=f"prow{j}", bufs=1)
                    nc.scalar.activation(prow[:, 0:512], pp0[:1, :], Act.Copy)
                    nc.scalar.activation(
                        prow[:, 512 : D + 1], pp1[:1, 0:449], Act.Copy
                    )
                    nc.scalar.activation(
                        murow[:, j : j + 1], pp1[:1, 448:449], Act.Copy
                    )
                    nc.sync.dma_start(pooled_all[j : j + 1, :], prow[:])

                if j == BPC - 1:
                    # last sample: no next-sample matmuls to hide behind --
                    # emit inline so pool chunks interleave with the LN chain
                    emit_pool()
                else:
                    pending_pool = emit_pool

            # ---- transpose pooled rows (mu correction is folded into the
            #      W1 matmul as a rank-1 term, see w1s) ----
            pooledT = stats.tile([PD, NDC, BPC], bf16, tag="pT")
            for c in range(NDC):
                pst = psum.tile([128, 512], f32, tag="mm", name=f"pst{c}")
                nc.tensor.transpose(
                    pst[:PD, :BPC],
                    pooled_all[:, c * PD : (c + 1) * PD],
                    mlp_w["id4"][:],
                )
                nc.scalar.activation(pooledT[:, c, :], pst[:PD, :BPC], Act.Copy)

            # ---- MLP in transposed layout ----
            h1T = stats.tile([128, 4, BPC], bf16, tag="h1T")
            for m in range(4):
                ps = psum.tile([128, 512], f32, tag="mm", name=f"psm1{m}")
                for c in range(NDC):
                    nc.tensor.matmul(
                        ps[:, :BPC],
                        lhsT=mlp_w["w1"][:, c, m * 128 : (m + 1) * 128],
                        rhs=pooledT[:, c, :],
                        start=(c == 0), stop=False,
                    )
                # rank-1 mu correction: h1 += (-W1e @ ones) * mu
                nc.tensor.matmul(
                    ps[:, :BPC],
                    lhsT=mlp_w["w1s"][:, m * 128 : (m + 1) * 128],
                    rhs=murow[:, :],
                    start=False, stop=True,
                )
                nc.scalar.activation(
                    h1T[:, m, :], ps[:, :BPC], Act.Relu, bias=mlp_w["b1"][:, m : m + 1]
                )
            h2T = stats.tile([128, 2, BPC], bf16, tag="h2T")
            for m in range(2):
                ps = psum.tile([128, 512], f32, tag="mm", name=f"psm2{m}")
                for c in range(4):
                    nc.tensor.matmul(
                        ps[:, :BPC],
                        lhsT=mlp_w["w2"][:, c, m * 128 : (m + 1) * 128],
                        rhs=h1T[:, c, :],
                        start=(c == 0), stop=(c == 3),
                    )
                nc.scalar.activation(
                    h2T[:, m, :], ps[:, :BPC], Act.Relu, bias=mlp_w["b2"][:, m : m + 1]
                )
            h3T = stats.tile([128, 1, BPC], bf16, tag="h3T")
            ps = psum.tile([128, 512], f32, tag="mm", name="psm3")
            for c in range(2):
                nc.tensor.matmul(
                    ps[:, :BPC],
                    lhsT=mlp_w["w3"][:, c, :],
                    rhs=h2T[:, c, :],
                    start=(c == 0), stop=(c == 1),
                )
            nc.scalar.activation(
                h3T[:, 0, :], ps[:, :BPC], Act.Relu, bias=mlp_w["b3"][:, 0:1]
            )
            ps4 = psum.tile([128, 512], f32, tag="mm", name="psm4")
            nc.tensor.matmul(
                ps4[:10, :BPC], lhsT=mlp_w["w4"][:, :], rhs=h3T[:, 0, :],
                start=True, stop=True,
            )
            osb = stats.tile([10, BPC], f32, tag="osb")
            nc.scalar.activation(osb[:], ps4[:10, :BPC], Act.Identity, bias=mlp_w["b4"][:])
            nc.sync.dma_start(out_h[:], osb[:])

    nc.compile()
    return nc


def _get_nc():
    if "nc" not in _CACHE:
        _CACHE["nc"] = _build_nc()
    return _CACHE["nc"]


def host_prep(inputs):
    """Build the 8 per-core in_maps from the full inputs."""
    x = np.asarray(inputs["x"], np.float32)
    mask = np.asarray(inputs["mask"])
    Wq, bq = np.asarray(inputs["Wq"], np.float32), np.asarray(inputs["bq"], np.float32)
    Wk, bk = np.asarray(inputs["Wk"], np.float32), np.asarray(inputs["bk"], np.float32)
    Wv, bv = np.asarray(inputs["Wv"], np.float32), np.asarray(inputs["bv"], np.float32)
    ln_g, ln_b = np.asarray(inputs["ln_g"], np.float32), np.asarray(inputs["ln_b"], np.float32)
    W1, b1 = np.asarray(inputs["W1"], np.float32), np.asarray(inputs["b1"], np.float32)
    W2, b2 = np.asarray(inputs["W2"], np.float32), np.asarray(inputs["b2"], np.float32)
    W3, b3 = np.asarray(inputs["W3"], np.float32), np.asarray(inputs["b3"], np.float32)
    W4, b4 = np.asarray(inputs["W4"], np.float32), np.asarray(inputs["b4"], np.float32)

    isq = 1.0 / np.sqrt(np.float32(D))
    # fused QK matrix, scaled into e4m3 range
    M = (Wq.T @ Wk) * isq
    Mpad = np.zeros((DP, DP), np.float32)
    Mpad[:D, :D] = M * SC_M
    m8 = Mpad.astype(F8)
    # key-side score bias direction (bq . K_k term)
    u_k = (Wk.T @ bq) * isq
    wvp = np.zeros((DP, D + 1), np.float32)
    wvp[:D, :D] = Wv.T * SC_V
    wvp[:D, D] = (Wv.T * SC_V).sum(axis=1)   # vsum col -> sum_d V[k,d]
    wv8 = wvp.astype(F8)

    W1e = W1 * ln_g[None, :]
    b1e = b1 + W1 @ ln_b
    w1s = np.ascontiguousarray((-W1e.sum(axis=1)).reshape(1, 512)).astype(BF16)
    w1 = np.ascontiguousarray(W1e.T).astype(BF16)
    b1p = np.ascontiguousarray(b1e.reshape(4, 128).T).astype(np.float32)
    w2 = np.ascontiguousarray(W2.T).astype(BF16)
    b2p = np.ascontiguousarray(b2.reshape(2, 128).T).astype(np.float32)
    w3 = np.ascontiguousarray(W3.T).astype(BF16)
    b3p = np.ascontiguousarray(b3.reshape(1, 128).T).astype(np.float32)
    w4 = np.ascontiguousarray(W4.T).astype(BF16)
    b4p = np.ascontiguousarray(b4.reshape(10, 1)).astype(np.float32)
    id4 = np.eye(4, dtype=np.float32)

    shared = dict(
        m8=m8, wv=wv8,
        w1s=w1s, w1=w1, w2=w2, w3=w3, w4=w4,
        b1=b1p, b2=b2p, b3=b3p, b4=b4p, id4=id4,
    )
    in_maps = []
    for core in range(NCORES):
        xt = np.zeros((BPC, DP, SP), F8)
        xn = np.zeros((BPC, SP, D), BF16)
        xs = np.zeros((BPC, 128, NKT), np.float32)
        mnp = np.full((BPC, 128, NKT), -1e9, np.float32)
        mfs = np.zeros((BPC, 128, NKT), np.float32)
        for jj in range(BPC):
            b = core * BPC + jj
            idx = np.nonzero(mask[b])[0]
            n = len(idx)
            assert n <= SP, f"sample {b}: {n} kept positions > SP={SP}"
            xk = x[b, idx]                                # [n, D]
            xt[jj, :D, :n] = xk.T.astype(F8)
            xnj = (xk + bv[None, :]).astype(BF16)
            xn[jj, :n] = xnj
            xsj = np.zeros(SP, np.float32)
            xsj[:n] = xnj.astype(np.float32).sum(axis=1)
            xs[jj] = xsj.reshape(NKT, 128).T
            mnpj = np.full(SP, -1e9, np.float32)
            mnpj[:n] = xk @ u_k
            mnp[jj] = mnpj.reshape(NKT, 128).T
            mfsj = np.zeros(SP, np.float32)
            mfsj[:n] = 1.0 / n
            mfs[jj] = mfsj.reshape(NKT, 128).T
        m = dict(shared)
        m.update(xt=xt, xn=xn, xs=xs, mnp=mnp, mfs=mfs)
        in_maps.append(m)
    return in_maps


def assemble(results):
    """results: list of 8 dicts with 'out' [10, BPC] -> [32, 10] f32."""
    return np.concatenate(
        [np.asarray(r["out"], np.float32).T for r in results], axis=0
    )


def kernel(**inputs):
    from concourse.bass_utils import run_bass_kernel_spmd

    nc = _get_nc()
    in_maps = host_prep(inputs)
    res = run_bass_kernel_spmd(nc, in_maps, core_ids=list(range(NCORES)))
    return assemble(res.results)


# revision 44
# speedup vs baseline: 1.0894x; 1.0333x over previous
"""Trainium2 Bass kernel for DeepProteinClassifier.

Contract: kernel(**inputs) takes the FULL unsharded inputs and returns
the FULL [32, 10] float32 output.

Sharding: data-parallel over batch B=32 across 8 NeuronCores (4 samples
per core); all weights replicated.

Optimizations over the naive formulation:
- Mask compaction: the output only depends on positions with mask==1
  (masked keys get exp(-1e9)=0 weight; the mean-pool zeroes masked
  queries). Each sample's ~487..543 kept positions are compacted and
  zero-padded to SP=640 (5 tiles of 128), cutting all attention-side
  work ~1.6x with bit-identical math for kept positions.
- Fused QK: scores = Q.K^T/sqrt(D) = x M x^T + (x.Wk^T bq)_k + c_q + c
  with M = Wq^T Wk / sqrt(D). Per-query constants cancel in softmax;
  the per-key term is a host-computed bias folded into the exp bias.
  This deletes one full 960x960 projection and both Q/K bias drains.
- fp8 (e4m3) matmuls in DoubleRow mode (2 contraction chunks per
  instruction = 2x PE throughput), fp32 PSUM accumulation. M is scaled
  by 1024 and Wv by 16 on host to stay in e4m3 normal range; scales are
  undone in the ACT drains. Softmax/LN statistics stay fp32, context
  and residual bf16.
- Contraction dim padded 960->1024 so all chunks are full 128 rows
  (4 DoubleRow pairs).

Per-core pipeline per sample (PE work interleaved so drains hide):
  T1T[j,q] = M^T-chunks @ xT-chunks (fp8 DR) -> DVE drain to fp8
  V[k,d|1] = xT-chunks^T @ Wv-chunks (fp8 DR) -> ACT drain (1/16) fp8
  ST[k,q]  = xT^T-chunks @ T1T-chunks (fp8 DR); ET = exp(ST/1024 + bias)
  CTX[q,d|r] = ET-chunks^T @ V-chunks (fp8 DR + 1 plain chunk)
  H = CTX/r + (x+bv); LN-stats; masked pool as PE matvec with
  alpha = mask/summask * rsqrt(var+eps); then 4-layer MLP (bf16).
"""

import numpy as np
import ml_dtypes

B, S, D = 32, 1024, 960
NCORES = 8
BPC = B // NCORES   # 4 samples per core
SP = 640            # kept positions per sample, padded (5 tiles of 128)
NKT = 5             # number of 128-row kept-position tiles
DP = 1024           # padded contraction dim (8 chunks of 128, 4 DR pairs)
NDC = 8             # number of 128-row d chunks
PD = 120            # MLP-side partition size (960 = 8*120)
LN_EPS = 1e-5
SC_M = 1024.0       # host scale on M (undone in exp drain)
SC_V = 16.0         # host scale on Wv (undone in V drain)
BF16 = ml_dtypes.bfloat16
F8 = ml_dtypes.float8_e4m3

_CACHE = {}


def _build_nc():
    import concourse.tile as tile
    from concourse import bacc, mybir

    class _Bacc(bacc.Bacc):
        """Bacc with the ACT table chooser steered to the combined
        ln+exp function set, so the per-sample Ln/Exp LayerNorm pair and
        the ET exp share ONE table (no per-sample ACT_TABLE_LOAD thrash)."""

        def insert_act_table_loads(self):
            import bass_rust as _bass_rust
            from concourse.hw_specs import get_activation_tables

            has_activation = any(
                isinstance(i, mybir.InstActivation)
                for b in self.main_func.blocks
                for i in b.instructions
            )
            if not has_activation:
                return
            tables = list(get_activation_tables(self.m.arch).items())
            combo = next(
                (f for n, f in tables if n == "natural_log_exp_and_others"), None
            )
            if combo is not None:
                tables = [
                    (n, f if n == "natural_log_exp_and_others" else f - combo)
                    for n, f in tables
                ]
            _bass_rust.insert_act_table_loads(self, tables)

    f32 = mybir.dt.float32
    bf16 = mybir.dt.bfloat16
    f8 = mybir.dt.float8e4
    Alu = mybir.AluOpType
    Act = mybir.ActivationFunctionType
    DR = mybir.MatmulPerfMode.DoubleRow

    nc = _Bacc("TRN2", target_bir_lowering=False, debug=False)

    # ---- DRAM parameters (per-core shard) ----
    xt_h = nc.declare_dram_parameter("xt", [BPC, DP, SP], f8, isOutput=False)
    xn_h = nc.declare_dram_parameter("xn", [BPC, SP, D], bf16, isOutput=False)
    xs_h = nc.declare_dram_parameter("xs", [BPC, 128, NKT], f32, isOutput=False)
    mnp_h = nc.declare_dram_parameter("mnp", [BPC, 128, NKT], f32, isOutput=False)
    mfs_h = nc.declare_dram_parameter("mfs", [BPC, 128, NKT], f32, isOutput=False)
    m8_h = nc.declare_dram_parameter("m8", [DP, DP], f8, isOutput=False)
    # wv col 960 = rowsums of Wv^T: the V projection then also emits
    # vsum_k = sum_d V[k,d], so sum_d ctx falls out of the ctx matmul as an
    # extra PSUM column (no ACT accumulators needed for the LN mean)
    wv_h = nc.declare_dram_parameter("wv", [DP, D + 1], f8, isOutput=False)
    w1s_h = nc.declare_dram_parameter("w1s", [1, 512], bf16, isOutput=False)
    w1_h = nc.declare_dram_parameter("w1", [D, 512], bf16, isOutput=False)
    w2_h = nc.declare_dram_parameter("w2", [512, 256], bf16, isOutput=False)
    w3_h = nc.declare_dram_parameter("w3", [256, 128], bf16, isOutput=False)
    w4_h = nc.declare_dram_parameter("w4", [128, 10], bf16, isOutput=False)
    b1_h = nc.declare_dram_parameter("b1", [128, 4], f32, isOutput=False)
    b2_h = nc.declare_dram_parameter("b2", [128, 2], f32, isOutput=False)
    b3_h = nc.declare_dram_parameter("b3", [128, 1], f32, isOutput=False)
    b4_h = nc.declare_dram_parameter("b4", [10, 1], f32, isOutput=False)
    id4_h = nc.declare_dram_parameter("id4", [4, 4], f32, isOutput=False)
    out_h = nc.declare_dram_parameter("out", [10, BPC], f32, isOutput=True)

    with tile.TileContext(nc) as tc:
        with (
            tc.tile_pool(name="wpool", bufs=1) as wpool,
            tc.tile_pool(name="xpool", bufs=2) as xpool,
            tc.tile_pool(name="big", bufs=2) as big,
            tc.tile_pool(name="stats", bufs=2) as stats,
            tc.tile_pool(name="psum", bufs=8, space="PSUM") as psum,
        ):
            def load_sample(j, defer=False):
                xt_sb = xpool.tile([128, NDC, SP], f8, tag="xt", name=f"xt{j}")
                if defer:
                    # pair-granular so the first T1T matmuls start ~1.5us in
                    for p in range(4):
                        nc.sync.dma_start(
                            xt_sb[:, 2 * p : 2 * p + 2, :],
                            xt_h[j, 256 * p : 256 * (p + 1)].rearrange(
                                "(c p) s -> p c s", p=128
                            ),
                        )
                else:
                    nc.sync.dma_start(
                        xt_sb[:], xt_h[j].rearrange("(c p) s -> p c s", p=128)
                    )
                xn_sb = xpool.tile([128, NKT, D], bf16, tag="xn", name=f"xn{j}")
                xs_sb = stats.tile([128, NKT], f32, tag="xs", name=f"xs{j}")
                mnp_sb = stats.tile([128, NKT], f32, tag="mnp", name=f"mnp{j}")
                mfs_sb = stats.tile([128, NKT], f32, tag="mfs", name=f"mfs{j}")
                if not defer:
                    nc.sync.dma_start(
                        xn_sb[:], xn_h[j].rearrange("(t p) d -> p t d", p=128)
                    )
                    nc.sync.dma_start(xs_sb[:], xs_h[j])
                    nc.sync.dma_start(mnp_sb[:], mnp_h[j])
                    nc.sync.dma_start(mfs_sb[:], mfs_h[j])
                return xt_sb, xn_sb, xs_sb, mnp_sb, mfs_sb

            # xt0 + attention weights first (they gate the first matmuls);
            # weight DMAs split in halves matching first-consumer slices so
            # compute starts as soon as each half lands. xn0/stats0 after.
            sample0 = load_sample(0, defer=True)
            # weights fetched on the Activation HWDGE queue, in parallel with
            # the sample loads on the sync queue
            m8_sb = wpool.tile([128, NDC, DP], f8)
            wv_sb = wpool.tile([128, NDC, DP], f8)
            nc.scalar.dma_start(
                m8_sb[:, :, 0:256],
                m8_h[:, 0:256].rearrange("(c p) n -> p c n", p=128),
            )
            nc.scalar.dma_start(
                m8_sb[:, :, 256:512],
                m8_h[:, 256:512].rearrange("(c p) n -> p c n", p=128),
            )
            nc.scalar.dma_start(
                wv_sb[:, :, 0:512],
                wv_h[:, 0:512].rearrange("(c p) n -> p c n", p=128),
            )
            nc.scalar.dma_start(
                m8_sb[:, :, 512:DP],
                m8_h[:, 512:DP].rearrange("(c p) n -> p c n", p=128),
            )
            nc.scalar.dma_start(
                wv_sb[:, :, 512 : D + 1],
                wv_h[:, 512 : D + 1].rearrange("(c p) n -> p c n", p=128),
            )
            nc.sync.dma_start(
                sample0[1][:], xn_h[0].rearrange("(t p) d -> p t d", p=128)
            )
            nc.sync.dma_start(sample0[2][:], xs_h[0])
            nc.sync.dma_start(sample0[3][:], mnp_h[0])
            nc.sync.dma_start(sample0[4][:], mfs_h[0])
            pooled_all = wpool.tile([BPC, D + 1], f32)
            murow = wpool.tile([1, BPC], bf16)
            mlp_w = {}

            def load_mlp_weights():
                w1s_sb = wpool.tile([1, 512], bf16, name="w1s_sb")
                nc.sync.dma_start(w1s_sb[:], w1s_h[:])
                mlp_w["w1s"] = w1s_sb
                w1_sb = wpool.tile([PD, NDC, 512], bf16, name="w1_sb")
                nc.sync.dma_start(w1_sb[:], w1_h[:].rearrange("(c p) n -> p c n", p=PD))
                w2_sb = wpool.tile([128, 4, 256], bf16, name="w2_sb")
                nc.sync.dma_start(w2_sb[:], w2_h[:].rearrange("(c p) n -> p c n", p=128))
                w3_sb = wpool.tile([128, 2, 128], bf16, name="w3_sb")
                nc.sync.dma_start(w3_sb[:], w3_h[:].rearrange("(c p) n -> p c n", p=128))
                w4_sb = wpool.tile([128, 10], bf16, name="w4_sb")
                nc.sync.dma_start(w4_sb[:], w4_h[:])
                b1_sb = wpool.tile([128, 4], f32, name="b1_sb")
                nc.sync.dma_start(b1_sb[:], b1_h[:])
                b2_sb = wpool.tile([128, 2], f32, name="b2_sb")
                nc.sync.dma_start(b2_sb[:], b2_h[:])
                b3_sb = wpool.tile([128, 1], f32, name="b3_sb")
                nc.sync.dma_start(b3_sb[:], b3_h[:])
                b4_sb = wpool.tile([10, 1], f32, name="b4_sb")
                nc.sync.dma_start(b4_sb[:], b4_h[:])
                id4_sb = wpool.tile([4, 4], f32, name="id4_sb")
                nc.sync.dma_start(id4_sb[:], id4_h[:])
                mlp_w.update(w1=w1_sb, w2=w2_sb, w3=w3_sb, w4=w4_sb,
                             b1=b1_sb, b2=b2_sb, b3=b3_sb, b4=b4_sb, id4=id4_sb)

            pending_pool = None

            for j in range(BPC):
                if j == 0:
                    xt_sb, xn_sb, xs_sb, mnp_sb, mfs_sb = sample0
                else:
                    xt_sb, xn_sb, xs_sb, mnp_sb, mfs_sb = load_sample(j)
                if j == 1:
                    load_mlp_weights()

                # ---- T1T = M^T-chunks @ xT-chunks: [do(1024), q(640)] fp8 ----
                T1T = big.tile([128, NDC, SP], f8, tag="T1T", name=f"T1T{j}")
                V = big.tile([128, NKT, 1024], f8, tag="V", name=f"V{j}")
                nc.vector.memset(V[:, :, D + 1 : D + 2], 1.0)

                def t1_chunks(ts, te):
                  with nc.named_scope(f"s{j}_t1"):
                    for t in range(ts, te):
                        psA = psum.tile([128, 512], f32, tag="mm", name="pt1a")
                        psB = psum.tile([128, 512], f32, tag="mm", name="pt1b")
                        for p in range(4):
                            lw = m8_sb[:, 2 * p : 2 * p + 2, t * 128 : (t + 1) * 128]
                            nc.tensor.matmul(
                                psA[:], lhsT=lw,
                                rhs=xt_sb[:, 2 * p : 2 * p + 2, 0:512],
                                start=(p == 0), stop=(p == 3), perf_mode=DR,
                            )
                            nc.tensor.matmul(
                                psB[:, 0:128], lhsT=lw,
                                rhs=xt_sb[:, 2 * p : 2 * p + 2, 512:SP],
                                start=(p == 0), stop=(p == 3), perf_mode=DR,
                            )
                        nc.vector.tensor_copy(T1T[:, t, 0:512], psA[:])
                        nc.vector.tensor_copy(T1T[:, t, 512:SP], psB[:, 0:128])

                def v_half(lo, hi):
                  with nc.named_scope(f"s{j}_v"):
                    for st in range(NKT):
                        ps = psum.tile([128, 512], f32, tag="mm", name="psv")
                        for p in range(4):
                            lx = xt_sb[:, 2 * p : 2 * p + 2, st * 128 : (st + 1) * 128]
                            nc.tensor.matmul(
                                ps[:, 0 : hi - lo], lhsT=lx,
                                rhs=wv_sb[:, 2 * p : 2 * p + 2, lo:hi],
                                start=(p == 0), stop=(p == 3), perf_mode=DR,
                            )
                        nc.vector.tensor_scalar_mul(
                            V[:, st, lo:hi], ps[:, 0 : hi - lo], 1.0 / SC_V
                        )

                # T1T t0-3 needs only the first m8 half; V's first half then
                # runs while the later weight-DMA halves land
                t1_chunks(0, 4)
                v_half(0, 512)
                t1_chunks(4, NDC)
                v_half(512, D + 1)

                # ---- ST = xT^T @ T1T; ET = exp(ST/1024 + keybias) fp8 ----
                ET = big.tile([128, NKT, SP], f8, tag="ET", name=f"ET{j}")
                with nc.named_scope(f"s{j}_st"):
                    for kt in range(NKT):
                        psA = psum.tile([128, 512], f32, tag="mm", name="pssa")
                        psB = psum.tile([128, 512], f32, tag="mm", name="pssb")
                        for p in range(4):
                            lx = xt_sb[:, 2 * p : 2 * p + 2, kt * 128 : (kt + 1) * 128]
                            nc.tensor.matmul(
                                psA[:], lhsT=lx,
                                rhs=T1T[:, 2 * p : 2 * p + 2, 0:512],
                                start=(p == 0), stop=(p == 3), perf_mode=DR,
                            )
                            nc.tensor.matmul(
                                psB[:, 0:128], lhsT=lx,
                                rhs=T1T[:, 2 * p : 2 * p + 2, 512:SP],
                                start=(p == 0), stop=(p == 3), perf_mode=DR,
                            )
                        nc.scalar.activation(
                            ET[:, kt, 0:512], psA[:], Act.Exp,
                            bias=mnp_sb[:, kt : kt + 1], scale=1.0 / SC_M,
                        )
                        nc.scalar.activation(
                            ET[:, kt, 512:SP], psB[:, 0:128], Act.Exp,
                            bias=mnp_sb[:, kt : kt + 1], scale=1.0 / SC_M,
                        )

                # previous sample's pool matvec lands here: its AL/H are long
                # ready, and it fills the PE while the ET exp drains finish
                if pending_pool is not None:
                    pending_pool()
                    pending_pool = None

                # ---- context + residual + per-tile LN stats (LayerNorm is
                #      per-row, so tile qt's alpha is ready as soon as its
                #      context drains -- the pool matvec pipelines per-tile) --
                H = big.tile([128, NKT, 1024], bf16, tag="H", name=f"H{j}")
                SQ = stats.tile([128, NKT], f32, tag="SQ", name=f"SQ{j}")
                recips = stats.tile([128, NKT], f32, tag="recips", name=f"rc{j}")
                MU = stats.tile([128, NKT], f32, tag="MU", name=f"MU{j}")
                VAR = stats.tile([128, NKT], f32, tag="VAR", name=f"VAR{j}")
                RS = stats.tile([128, NKT], f32, tag="RS", name=f"RS{j}")
                AL = stats.tile([128, NKT], bf16, tag="AL", name=f"AL{j}")
                with nc.named_scope(f"s{j}_ctx"):
                    for qt in range(NKT):
                        ps0 = psum.tile([128, 512], f32, tag="mm", name="psc0")
                        ps1 = psum.tile([128, 512], f32, tag="mm", name="psc1")
                        for p in range(2):
                            le = ET[:, 2 * p : 2 * p + 2, qt * 128 : (qt + 1) * 128]
                            nc.tensor.matmul(
                                ps0[:], lhsT=le,
                                rhs=V[:, 2 * p : 2 * p + 2, 0:512],
                                start=(p == 0), stop=False, perf_mode=DR,
                            )
                            nc.tensor.matmul(
                                ps1[:, 0:450], lhsT=le,
                                rhs=V[:, 2 * p : 2 * p + 2, 512 : D + 2],
                                start=(p == 0), stop=False, perf_mode=DR,
                            )
                        le = ET[:, 4, qt * 128 : (qt + 1) * 128]
                        nc.tensor.matmul(
                            ps0[:], lhsT=le, rhs=V[:, 4, 0:512],
                            start=False, stop=True,
                        )
                        nc.tensor.matmul(
                            ps1[:, 0:450], lhsT=le, rhs=V[:, 4, 512 : D + 2],
                            start=False, stop=True,
                        )
                        # col 449: r (softmax denom); col 448: sum_d ctx_d
                        q = slice(qt, qt + 1)
                        nc.vector.reciprocal(
                            recips[:, q], ps1[:, 449:450]
                        )
                        ctx0 = stats.tile([128, 512], bf16, tag="ctx0",
                                          name=f"c0_{j}_{qt}")
                        ctx1 = stats.tile([128, 448], bf16, tag="ctx1",
                                          name=f"c1_{j}_{qt}")
                        nc.scalar.activation(
                            ctx0[:], ps0[:], Act.Copy,
                            scale=recips[:, q],
                        )
                        nc.vector.tensor_scalar(
                            ctx1[:], ps1[:, 0:448], recips[:, q], None,
                            Alu.mult, Alu.bypass,
                        )
                        # mu = (sum ctx + sum xn)/D, from the vsum PSUM column
                        nc.vector.tensor_tensor(MU[:, q], ps1[:, 448:449],
                                                recips[:, q], Alu.mult)
                        nc.vector.tensor_scalar(
                            MU[:, q], MU[:, q], xs_sb[:, q], 1.0 / D,
                            Alu.add, Alu.mult,
                        )
                        nc.vector.tensor_copy(H[:, qt, D : D + 1], MU[:, q])
                        nc.vector.tensor_add(
                            H[:, qt, 0:512], ctx0[:], xn_sb[:, qt, 0:512]
                        )
                        nc.vector.tensor_add(
                            H[:, qt, 512:D], ctx1[:], xn_sb[:, qt, 512:D]
                        )
                        scratch = stats.tile(
                            [128, D], bf16, tag="scr", name=f"scr{j}_{qt}", bufs=1
                        )
                        nc.scalar.activation(
                            scratch[:], H[:, qt, 0:D], Act.Square,
                            accum_out=SQ[:, qt : qt + 1],
                        )
                        # var = SQ/D + eps - mu^2 -> rs = exp(-0.5 ln var)
                        nc.vector.tensor_tensor(VAR[:, q], MU[:, q], MU[:, q],
                                                Alu.mult)
                        T2q = stats.tile([128, 1], f32, tag="T2",
                                         name=f"T2{j}_{qt}")
                        nc.vector.tensor_scalar(
                            T2q[:], SQ[:, q], 1.0 / D, LN_EPS,
                            Alu.mult, Alu.add,
                        )
                        nc.vector.tensor_sub(VAR[:, q], T2q[:], VAR[:, q])
                        nc.scalar.activation(VAR[:, q], VAR[:, q], Act.Ln)
                        nc.scalar.activation(RS[:, q], VAR[:, q], Act.Exp,
                                             scale=-0.5)
                        nc.vector.tensor_tensor(AL[:, q], mfs_sb[:, q],
                                                RS[:, q], Alu.mult)

                # ---- masked-mean pool as PE matvec; each chunk c waits only
                #      on its own AL column so it pipelines with the LN chain
                def emit_pool(j=j, AL=AL, H=H):
                    pp0 = psum.tile([128, 512], f32, tag="mm", name="pp0")
                    pp1 = psum.tile([128, 512], f32, tag="mm", name="pp1")
                    for c in range(NKT):
                        nc.tensor.matmul(
                            pp0[:1, :],
                            lhsT=AL[:, c : c + 1],
                            rhs=H[:, c, 0:512],
                            start=(c == 0), stop=(c == NKT - 1),
                        )
                        nc.tensor.matmul(
                            pp1[:1, 0:449],
                            lhsT=AL[:, c : c + 1],
                            rhs=H[:, c, 512 : D + 1],
                            start=(c == 0), stop=(c == NKT - 1),
                        )
                    prow = stats.tile([1, D + 1], f32, tag="prow",
                                      name=f"prow{j}", bufs=1)
                    nc.scalar.activation(prow[:, 0:512], pp0[:1, :], Act.Copy)
                    nc.scalar.activation(
                        prow[:, 512 : D + 1], pp1[:1, 0:449], Act.Copy
                    )
                    nc.scalar.activation(
                        murow[:, j : j + 1], pp1[:1, 448:449], Act.Copy
                    )
                    nc.sync.dma_start(pooled_all[j : j + 1, :], prow[:])

                if j == BPC - 1:
                    # last sample: no next-sample matmuls to hide behind --
                    # emit inline so pool chunks interleave with the LN chain
                    emit_pool()
                else:
                    pending_pool = emit_pool

            # ---- transpose pooled rows (mu correction is folded into the
            #      W1 matmul as a rank-1 term, see w1s) ----
            pooledT = stats.tile([PD, NDC, BPC], bf16, tag="pT")
            for c in range(NDC):
                pst = psum.tile([128, 512], f32, tag="mm", name=f"pst{c}")
                nc.tensor.transpose(
                    pst[:PD, :BPC],
                    pooled_all[:, c * PD : (c + 1) * PD],
                    mlp_w["id4"][:],
                )
                nc.scalar.activation(pooledT[:, c, :], pst[:PD, :BPC], Act.Copy)

            # ---- MLP in transposed layout ----
            h1T = stats.tile([128, 4, BPC], bf16, tag="h1T")
            for m in range(4):
                ps = psum.tile([128, 512], f32, tag="mm", name=f"psm1{m}")
                for c in range(NDC):
                    nc.tensor.matmul(
                        ps[:, :BPC],
                        lhsT=mlp_w["w1"][:, c, m * 128 : (m + 1) * 128],
                        rhs=pooledT[:, c, :],
                        start=(c == 0), stop=False,
                    )
                # rank-1 mu correction: h1 += (-W1e @ ones) * mu
                nc.tensor.matmul(
                    ps[:, :BPC],
                    lhsT=mlp_w["w1s"][:, m * 128 : (m + 1) * 128],
                    rhs=murow[:, :],
                    start=False, stop=True,
                )
                nc.scalar.activation(
                    h1T[:, m, :], ps[:, :BPC], Act.Relu, bias=mlp_w["b1"][:, m : m + 1]
                )
            h2T = stats.tile([128, 2, BPC], bf16, tag="h2T")
            for m in range(2):
                ps = psum.tile([128, 512], f32, tag="mm", name=f"psm2{m}")
                for c in range(4):
                    nc.tensor.matmul(
                        ps[:, :BPC],
                        lhsT=mlp_w["w2"][:, c, m * 128 : (m + 1) * 128],
                        rhs=h1T[:, c, :],
                        start=(c == 0), stop=(c == 3),
                    )
                nc.scalar.activation(
                    h2T[:, m, :], ps[:, :BPC], Act.Relu, bias=mlp_w["b2"][:, m : m + 1]
                )
            h3T = stats.tile([128, 1, BPC], bf16, tag="h3T")
            ps = psum.tile([128, 512], f32, tag="mm", name="psm3")
            for c in range(2):
                nc.tensor.matmul(
                    ps[:, :BPC],
                    lhsT=mlp_w["w3"][:, c, :],
                    rhs=h2T[:, c, :],
                    start=(c == 0), stop=(c == 1),
                )
            nc.scalar.activation(
                h3T[:, 0, :], ps[:, :BPC], Act.Relu, bias=mlp_w["b3"][:, 0:1]
            )
            ps4 = psum.tile([128, 512], f32, tag="mm", name="psm4")
            nc.tensor.matmul(
                ps4[:10, :BPC], lhsT=mlp_w["w4"][:, :], rhs=h3T[:, 0, :],
                start=True, stop=True,
            )
            osb = stats.tile([10, BPC], f32, tag="osb")
            nc.scalar.activation(osb[:], ps4[:10, :BPC], Act.Identity, bias=mlp_w["b4"][:])
            nc.sync.dma_start(out_h[:], osb[:])

    nc.compile()
    return nc


def _get_nc():
    if "nc" not in _CACHE:
        _CACHE["nc"] = _build_nc()
    return _CACHE["nc"]


def host_prep(inputs):
    """Build the 8 per-core in_maps from the full inputs."""
    x = np.asarray(inputs["x"], np.float32)
    mask = np.asarray(inputs["mask"])
    Wq, bq = np.asarray(inputs["Wq"], np.float32), np.asarray(inputs["bq"], np.float32)
    Wk, bk = np.asarray(inputs["Wk"], np.float32), np.asarray(inputs["bk"], np.float32)
    Wv, bv = np.asarray(inputs["Wv"], np.float32), np.asarray(inputs["bv"], np.float32)
    ln_g, ln_b = np.asarray(inputs["ln_g"], np.float32), np.asarray(inputs["ln_b"], np.float32)
    W1, b1 = np.asarray(inputs["W1"], np.float32), np.asarray(inputs["b1"], np.float32)
    W2, b2 = np.asarray(inputs["W2"], np.float32), np.asarray(inputs["b2"], np.float32)
    W3, b3 = np.asarray(inputs["W3"], np.float32), np.asarray(inputs["b3"], np.float32)
    W4, b4 = np.asarray(inputs["W4"], np.float32), np.asarray(inputs["b4"], np.float32)

    isq = 1.0 / np.sqrt(np.float32(D))
    # fused QK matrix, scaled into e4m3 range
    M = (Wq.T @ Wk) * isq
    Mpad = np.zeros((DP, DP), np.float32)
    Mpad[:D, :D] = M * SC_M
    m8 = Mpad.astype(F8)
    # key-side score bias direction (bq . K_k term)
    u_k = (Wk.T @ bq) * isq
    wvp = np.zeros((DP, D + 1), np.float32)
    wvp[:D, :D] = Wv.T * SC_V
    wvp[:D, D] = (Wv.T * SC_V).sum(axis=1)   # vsum col -> sum_d V[k,d]
    wv8 = wvp.astype(F8)

    W1e = W1 * ln_g[None, :]
    b1e = b1 + W1 @ ln_b
    w1s = np.ascontiguousarray((-W1e.sum(axis=1)).reshape(1, 512)).astype(BF16)
    w1 = np.ascontiguousarray(W1e.T).astype(BF16)
    b1p = np.ascontiguousarray(b1e.reshape(4, 128).T).astype(np.float32)
    w2 = np.ascontiguousarray(W2.T).astype(BF16)
    b2p = np.ascontiguousarray(b2.reshape(2, 128).T).astype(np.float32)
    w3 = np.ascontiguousarray(W3.T).astype(BF16)
    b3p = np.ascontiguousarray(b3.reshape(1, 128).T).astype(np.float32)
    w4 = np.ascontiguousarray(W4.T).astype(BF16)
    b4p = np.ascontiguousarray(b4.reshape(10, 1)).astype(np.float32)
    id4 = np.eye(4, dtype=np.float32)

    shared = dict(
        m8=m8, wv=wv8,
        w1s=w1s, w1=w1, w2=w2, w3=w3, w4=w4,
        b1=b1p, b2=b2p, b3=b3p, b4=b4p, id4=id4,
    )
    in_maps = []
    for core in range(NCORES):
        xt = np.zeros((BPC, DP, SP), F8)
        xn = np.zeros((BPC, SP, D), BF16)
        xs = np.zeros((BPC, 128, NKT), np.float32)
        mnp = np.full((BPC, 128, NKT), -1e9, np.float32)
        mfs = np.zeros((BPC, 128, NKT), np.float32)
        for jj in range(BPC):
            b = core * BPC + jj
            idx = np.nonzero(mask[b])[0]
            n = len(idx)
            assert n <= SP, f"sample {b}: {n} kept positions > SP={SP}"
            xk = x[b, idx]                                # [n, D]
            xt[jj, :D, :n] = xk.T.astype(F8)
            xnj = (xk + bv[None, :]).astype(BF16)
            xn[jj, :n] = xnj
            xsj = np.zeros(SP, np.float32)
            xsj[:n] = xnj.astype(np.float32).sum(axis=1)
            xs[jj] = xsj.reshape(NKT, 128).T
            mnpj = np.full(SP, -1e9, np.float32)
            mnpj[:n] = xk @ u_k
            mnp[jj] = mnpj.reshape(NKT, 128).T
            mfsj = np.zeros(SP, np.float32)
            mfsj[:n] = 1.0 / n
            mfs[jj] = mfsj.reshape(NKT, 128).T
        m = dict(shared)
        m.update(xt=xt, xn=xn, xs=xs, mnp=mnp, mfs=mfs)
        in_maps.append(m)
    return in_maps


def assemble(results):
    """results: list of 8 dicts with 'out' [10, BPC] -> [32, 10] f32."""
    return np.concatenate(
        [np.asarray(r["out"], np.float32).T for r in results], axis=0
    )


def kernel(**inputs):
    from concourse.bass_utils import run_bass_kernel_spmd

    nc = _get_nc()
    in_maps = host_prep(inputs)
    res = run_bass_kernel_spmd(nc, in_maps, core_ids=list(range(NCORES)))
    return assemble(res.results)
